# revision 4
# baseline (speedup 1.0000x reference)
"""Trainium2 Bass kernel for nn_Model_1580547969651.

Math (from the reference):
    s    = x @ sum(y, axis=0)          # (B,) row-sums of x @ y^T
    h    = hardswish(s)                # s * clip(s+3, 0, 6) / 6
    out  = clip(h + noise, -0.5, 0.5)  # (B, 1)

Strategy (column-shard, single post-stream AllToAll):
  - x and y column-sharded (512 features/core); y streams first, x second,
    on both HWDGE queues; 2MB super-tiles with (s p c) packing.
  - Phase A folds y into acc entirely on the DVE (PE fp32 matmuls are ~2x
    slower than the stream); the LAST super-tile is DMA'd as four 0.5MB
    pair-chunks so its folds chase the arrivals, and one accumulating
    matmul broadcasts the column-sum into PSUM ~6us after the y stream.
  - Phase B: 64 scalar_tensor_tensor dots for all 8192 rows (DVE-serial,
    ~50us - the pacing engine of the back half). NOTE: tensor_tensor_reduce
    looks perfect here but HANGS TRN2 hardware (sim passes; v2/v3 died).
  - Partials go p-major straight to the collective bounce buffer (no
    transposes; host undoes the layout at gather time). ONE post-stream
    32KB AllToAll (Mesh; ReduceScatter picks RDH = documented hang
    suspect, and a mid-stream collective cannot start anyway: the ncfw
    stream is busy with the warm-up AR until past stream end, and any
    in-stream collective data-plane crawls 3x + hung once).
  - Each core folds the 8 received shards (= its own 1024 rows) on the
    DVE, runs the tiny elementwise tail, outputs (16, 64).
  - Warm-up 32B AllReduce up front initiates the ~67us ncfw wake during
    the streams.
"""

import numpy as np

from concourse import bass, bacc, mybir, tile
from concourse.bass_utils import run_bass_kernel_spmd

B = 8192
F = 4096
NCORES = 8
FL = F // NCORES        # 512 features per core
BL = B // NCORES        # 1024 output rows per core
NST = 8                 # super-tiles (128 part x 8 subtiles x 512)
NSUB = 8                # subtiles per super-tile
NT = NST * NSUB         # 64 (128-row) groups covering all 8192 rows
FP32 = mybir.dt.float32

_CACHE: dict = {}


def _build():
    nc = bacc.Bacc(
        "TRN2",
        target_bir_lowering=False,
        debug=False,
        num_devices=NCORES,
    )

    x_d = nc.dram_tensor("x", [B, FL], FP32, kind="ExternalInput")
    y_d = nc.dram_tensor("y", [B, FL], FP32, kind="ExternalInput")
    nz_d = nc.dram_tensor("noise", [16, NT], FP32, kind="ExternalInput")
    out_d = nc.dram_tensor("out", [16, NT], FP32, kind="ExternalOutput")

    # (s p c) packing: partition p's slice of super-tile s is 8 consecutive
    # DRAM rows = one contiguous 16KB chunk per descriptor.
    y_r = y_d[:, :].rearrange("(s p c) f -> s p c f", p=128, c=NSUB)
    x_r = x_d[:, :].rearrange("(s p c) f -> s p c f", p=128, c=NSUB)

    with tile.TileContext(nc) as tc:
        with (
            tc.tile_pool(name="ypool", bufs=5) as ypool,
            tc.tile_pool(name="xpool", bufs=5) as xpool,
            tc.tile_pool(name="small", bufs=1) as small,
            tc.tile_pool(name="scratch", bufs=3) as scratch,
            tc.tile_pool(name="psum", bufs=1, space="PSUM") as psum,
            tc.tile_pool(name="dram", bufs=1, space="DRAM") as dram,
        ):
            # warm-up collective FIRST: initiates the ~40us ncfw wake
            # immediately. AllToAll of 32B garbage (values unused): no
            # staging DMA before the trigger, and A2A occupies the ncfw
            # stream for less time than an AllReduce when entry skew is
            # large (it would otherwise delay the real A2A).
            warm_in = dram.tile([8], FP32)
            warm_out = dram.tile([8], FP32)
            nc.gpsimd.collective_compute(
                "AllToAll",
                mybir.AluOpType.bypass,
                replica_groups=[list(range(NCORES))],
                ins=[warm_in.opt()],
                outs=[warm_out.opt()],
            )

            ones128 = small.tile([128, 128], FP32)
            nc.gpsimd.memset(ones128[:], 1.0)

            # noise prefetch on the SWDGE queue (HWDGE queues stay pure)
            noise_t = small.tile([16, NT], FP32)
            nc.gpsimd.dma_start(noise_t[:], nz_d[:, :])

            # ---- phase A: stream y, fold into acc on the DVE ----
            acc = small.tile([128, FL], FP32)
            bc = psum.tile([128, FL], FP32, tag="bc")
            for s in range(NST):
                ytile = ypool.tile([128, NSUB, FL], FP32, tag="y")
                if s < NST - 1:
                    nc.sync.dma_start(ytile[:, 0:4, :], y_r[s, :, 0:4, :])
                    nc.scalar.dma_start(ytile[:, 4:8, :], y_r[s, :, 4:8, :])
                    nc.vector.tensor_add(ytile[:, 0:4, :], ytile[:, 0:4, :],
                                         ytile[:, 4:8, :])
                    nc.vector.tensor_add(ytile[:, 0:2, :], ytile[:, 0:2, :],
                                         ytile[:, 2:4, :])
                    if s == 0:
                        nc.vector.tensor_tensor(
                            out=acc[:], in0=ytile[:, 0, :],
                            in1=ytile[:, 1, :], op=mybir.AluOpType.add)
                    else:
                        nc.vector.tensor_add(ytile[:, 0, :], ytile[:, 0, :],
                                             ytile[:, 1, :])
                        nc.vector.tensor_add(acc[:], acc[:], ytile[:, 0, :])
                else:
                    # last super-tile: wide folds (3 adds beat 7 narrow
                    # ones: per-op overhead dominates at 512 wide)
                    nc.sync.dma_start(ytile[:, 0:4, :], y_r[s, :, 0:4, :])
                    nc.scalar.dma_start(ytile[:, 4:8, :], y_r[s, :, 4:8, :])
                    nc.vector.tensor_add(ytile[:, 0:4, :], ytile[:, 0:4, :],
                                         ytile[:, 4:8, :])
                    nc.vector.tensor_add(ytile[:, 0:2, :], ytile[:, 0:2, :],
                                         ytile[:, 2:4, :])
                    nc.vector.tensor_add(ytile[:, 0, :], ytile[:, 0, :],
                                         ytile[:, 1, :])
                    nc.vector.tensor_add(acc[:], acc[:], ytile[:, 0, :])
            nc.tensor.matmul(bc[:], ones128[:], acc[:],
                             start=True, stop=True)

            # ---- phase B: partial dots for ALL rows while x streams ----
            s_part = small.tile([128, NT], FP32)
            cc_in = dram.tile([B], FP32)
            cc_out = dram.tile([B], FP32)
            for s in range(NST):
                xtile = xpool.tile([128, NSUB, FL], FP32, tag="x")
                cut = NSUB // 2 if s < NST - 1 else 6
                nc.sync.dma_start(xtile[:, 0:cut, :], x_r[s, :, 0:cut, :])
                nc.scalar.dma_start(xtile[:, cut:, :], x_r[s, :, cut:, :])
                for t in range(NSUB):
                    m = s * NSUB + t
                    prod = scratch.tile([128, FL], FP32, tag="sc")
                    nc.vector.scalar_tensor_tensor(
                        out=prod[:],
                        in0=xtile[:, t, :],
                        scalar=1.0,
                        in1=bc[:],
                        op0=mybir.AluOpType.mult,
                        op1=mybir.AluOpType.mult,
                        accum_out=s_part[:, m:m + 1],
                    )
                # bounce finished partial columns early so only a 4KB
                # chunk trails the final dot
                if s == 3:
                    nc.gpsimd.dma_start(
                        cc_in[:].rearrange("(p m) -> p m", p=128)[:, 0:32],
                        s_part[:, 0:32])
                elif s == 6:
                    nc.gpsimd.dma_start(
                        cc_in[:].rearrange("(p m) -> p m", p=128)[:, 32:56],
                        s_part[:, 32:56])

            # ---- ONE post-stream A2A of all partials (p-major) ----
            nc.gpsimd.dma_start(
                cc_in[:].rearrange("(p m) -> p m", p=128)[:, 56:64],
                s_part[:, 56:64])
            nc.gpsimd.collective_compute(
                "AllToAll",
                mybir.AluOpType.bypass,
                replica_groups=[list(range(NCORES))],
                ins=[cc_in.opt()],
                outs=[cc_out.opt()],
            )

            # ---- tail: fold 8 shards (own 1024 rows), hardswish, out ----
            # A2A out[k]: k = 1024*j + 64*pl + m -> (pl, j, m); element
            # (pl, m) after the j-fold is global row 128m + 16r + pl
            st = small.tile([16, NT], FP32)
            fa = small.tile([16, NSUB, NT], FP32, tag="fa")
            cc_or = cc_out[:].rearrange("(j pl m) -> pl j m", pl=16, m=NT)
            nc.sync.dma_start(fa[:, 0:4, :], cc_or[:, 0:4, :])
            nc.scalar.dma_start(fa[:, 4:8, :], cc_or[:, 4:8, :])
            nc.vector.tensor_add(fa[:, 0:4, :], fa[:, 0:4, :], fa[:, 4:8, :])
            nc.vector.tensor_add(fa[:, 0:2, :], fa[:, 0:2, :], fa[:, 2:4, :])
            nc.vector.tensor_tensor(
                out=st[:], in0=fa[:, 0, :], in1=fa[:, 1, :],
                op=mybir.AluOpType.add)
            t_ = small.tile([16, NT], FP32)
            nc.vector.tensor_scalar(
                out=t_[:], in0=st[:], scalar1=3.0, scalar2=0.0,
                op0=mybir.AluOpType.add, op1=mybir.AluOpType.max,
            )
            nc.vector.tensor_scalar(
                out=t_[:], in0=t_[:], scalar1=6.0, scalar2=1.0 / 6.0,
                op0=mybir.AluOpType.min, op1=mybir.AluOpType.mult,
            )
            r = small.tile([16, NT], FP32)
            nc.vector.tensor_tensor(
                out=r[:], in0=st[:], in1=t_[:], op=mybir.AluOpType.mult,
            )
            nc.vector.tensor_tensor(
                out=r[:], in0=r[:], in1=noise_t[:], op=mybir.AluOpType.add,
            )
            nc.vector.tensor_scalar(
                out=r[:], in0=r[:], scalar1=-0.5, scalar2=0.5,
                op0=mybir.AluOpType.max, op1=mybir.AluOpType.min,
            )
            nc.sync.dma_start(out_d[:, :], r[:])

    nc.compile()
    return nc


def _get_nc():
    if "nc" not in _CACHE:
        _CACHE["nc"] = _build()
    return _CACHE["nc"]


# device row (s p c) -> global row 128*(8s+c)+p, so that partials column
# m = 8s+c, partition p lands at global row 128m+p
def _permute_rows(a: np.ndarray) -> np.ndarray:
    # a: (8192, cols); view as (s, c, p, cols), want (s, p, c, cols)
    return np.ascontiguousarray(
        a.reshape(NST, NSUB, 128, a.shape[1]).transpose(0, 2, 1, 3)
        .reshape(B, a.shape[1])
    )


def kernel(x: np.ndarray, y: np.ndarray, noise: np.ndarray, **_run_kwargs) -> np.ndarray:
    x = np.ascontiguousarray(x, dtype=np.float32)
    y = np.ascontiguousarray(y, dtype=np.float32)
    noise = np.ascontiguousarray(noise, dtype=np.float32)

    nc = _get_nc()
    xp = _permute_rows(x)
    # noise for core r in (16, 64) layout: element (pl, m) = global row
    # 128m + 16r + pl
    nz = noise[:, 0].reshape(NT, 128).T     # (128, 64): (p, m)
    in_maps = [
        {
            "x": np.ascontiguousarray(xp[:, i * FL:(i + 1) * FL]),
            "y": np.ascontiguousarray(y[:, i * FL:(i + 1) * FL]),
            "noise": np.ascontiguousarray(nz[16 * i:16 * (i + 1), :]),
        }
        for i in range(NCORES)
    ]
    res = run_bass_kernel_spmd(nc, in_maps, list(range(NCORES)), **_run_kwargs)
    # core r's out (16, 64): element (pl, m) = global row 128m + 16r + pl
    stacked = np.stack([res.results[i]["out"] for i in range(NCORES)])  # (8,16,64)
    out = stacked.transpose(2, 0, 1).reshape(B, 1)                      # (64*8*16,1)
    if _run_kwargs:
        _CACHE["last_results"] = res
    return out


# revision 6
# speedup vs baseline: 1.3927x; 1.3927x over previous
"""Trainium2 Bass kernel for nn_Model_1580547969651.

Math (from the reference):
    s    = x @ sum(y, axis=0)          # (B,) row-sums of x @ y^T
    h    = hardswish(s)                # s * clip(s+3, 0, 6) / 6
    out  = clip(h + noise, -0.5, 0.5)  # (B, 1)

Strategy (column-shard, single post-stream AllToAll):
  - x and y column-sharded (512 features/core); y streams first, x second,
    on both HWDGE queues; 2MB super-tiles with (s p c) packing.
  - Phase A folds y into acc on the DVE (PE fp32 matmuls are throttle-
    sensitive and ~2x slower than the stream); the LAST super-tile's fold
    chain - the only one on the critical path - is column-split across
    DVE (0:384) and gpsimd (384:512) to run in parallel, then one
    matmul broadcasts the column-sum into PSUM ~4.5us after the y stream.
  - Phase B: 64 scalar_tensor_tensor dots for all 8192 rows (DVE-serial,
    ~50us - the pacing engine of the back half). NOTE: tensor_tensor_reduce
    looks perfect here but HANGS TRN2 hardware (sim passes; v2/v3 died).
  - Partials go p-major straight to the collective bounce buffer (no
    transposes; host undoes the layout at gather time). ONE post-stream
    32KB AllToAll (Mesh; ReduceScatter picks RDH = documented hang
    suspect, and a mid-stream collective cannot start anyway: the ncfw
    stream is busy with the warm-up AR until past stream end, and any
    in-stream collective data-plane crawls 3x + hung once).
  - Each core folds the 8 received shards (= its own 1024 rows) on the
    DVE, runs the tiny elementwise tail, outputs (16, 64).
  - Warm-up 32B AllToAll (garbage input, no staging DMA) is the first
    gpsimd op: initiates the ~40us ncfw wake immediately and occupies
    the serialized ncfw stream for less time than an AllReduce would.
"""

import numpy as np

from concourse import bass, bacc, mybir, tile
from concourse.bass_utils import run_bass_kernel_spmd

B = 8192
F = 4096
NCORES = 8
FL = F // NCORES        # 512 features per core
BL = B // NCORES        # 1024 output rows per core
NST = 8                 # super-tiles (128 part x 8 subtiles x 512)
NSUB = 8                # subtiles per super-tile
NT = NST * NSUB         # 64 (128-row) groups covering all 8192 rows
FP32 = mybir.dt.float32

_CACHE: dict = {}


def _build():
    nc = bacc.Bacc(
        "TRN2",
        target_bir_lowering=False,
        debug=False,
        num_devices=NCORES,
    )

    x_d = nc.dram_tensor("x", [B, FL], FP32, kind="ExternalInput")
    y_d = nc.dram_tensor("y", [B, FL], FP32, kind="ExternalInput")
    nz_d = nc.dram_tensor("noise", [16, NT], FP32, kind="ExternalInput")
    out_d = nc.dram_tensor("out", [16, NT], FP32, kind="ExternalOutput")

    # (s p c) packing: partition p's slice of super-tile s is 8 consecutive
    # DRAM rows = one contiguous 16KB chunk per descriptor.
    y_r = y_d[:, :].rearrange("(s p c) f -> s p c f", p=128, c=NSUB)
    x_r = x_d[:, :].rearrange("(s p c) f -> s p c f", p=128, c=NSUB)

    with tile.TileContext(nc) as tc:
        with (
            tc.tile_pool(name="ypool", bufs=5) as ypool,
            tc.tile_pool(name="xpool", bufs=5) as xpool,
            tc.tile_pool(name="small", bufs=1) as small,
            tc.tile_pool(name="scratch", bufs=3) as scratch,
            tc.tile_pool(name="psum", bufs=1, space="PSUM") as psum,
            tc.tile_pool(name="dram", bufs=1, space="DRAM") as dram,
        ):
            # warm-up collective FIRST: initiates the ~40us ncfw wake
            # immediately. AllToAll of 32B garbage (values unused): no
            # staging DMA before the trigger, and A2A occupies the ncfw
            # stream for less time than an AllReduce when entry skew is
            # large (it would otherwise delay the real A2A).
            warm_in = dram.tile([8], FP32)
            warm_out = dram.tile([8], FP32)
            nc.gpsimd.collective_compute(
                "AllToAll",
                mybir.AluOpType.bypass,
                replica_groups=[list(range(NCORES))],
                ins=[warm_in.opt()],
                outs=[warm_out.opt()],
            )

            ones128 = small.tile([128, 128], FP32)
            nc.gpsimd.memset(ones128[:], 1.0)

            # noise prefetch on the SWDGE queue (HWDGE queues stay pure)
            noise_t = small.tile([16, NT], FP32)
            nc.gpsimd.dma_start(noise_t[:], nz_d[:, :])

            # ---- phase A: stream y, fold into acc on the DVE ----
            acc = small.tile([128, FL], FP32)
            bc = psum.tile([128, FL], FP32, tag="bc")
            for s in range(NST):
                ytile = ypool.tile([128, NSUB, FL], FP32, tag="y")
                if s < NST - 1:
                    nc.sync.dma_start(ytile[:, 0:4, :], y_r[s, :, 0:4, :])
                    nc.scalar.dma_start(ytile[:, 4:8, :], y_r[s, :, 4:8, :])
                    nc.vector.tensor_add(ytile[:, 0:4, :], ytile[:, 0:4, :],
                                         ytile[:, 4:8, :])
                    nc.vector.tensor_add(ytile[:, 0:2, :], ytile[:, 0:2, :],
                                         ytile[:, 2:4, :])
                    if s == 0:
                        nc.vector.tensor_tensor(
                            out=acc[:], in0=ytile[:, 0, :],
                            in1=ytile[:, 1, :], op=mybir.AluOpType.add)
                    else:
                        nc.vector.tensor_add(ytile[:, 0, :], ytile[:, 0, :],
                                             ytile[:, 1, :])
                        nc.vector.tensor_add(acc[:], acc[:], ytile[:, 0, :])
                else:
                    # last super-tile's folds are on the critical path
                    # (everything before bc): split columns across the
                    # DVE (0:384) and gpsimd (384:512) so the two fold
                    # chains run in parallel (~3.8us instead of 4.9us)
                    nc.sync.dma_start(ytile[:, 0:4, :], y_r[s, :, 0:4, :])
                    nc.scalar.dma_start(ytile[:, 4:8, :], y_r[s, :, 4:8, :])
                    for eng, lo, hi in ((nc.vector, 0, 384),
                                        (nc.gpsimd, 384, FL)):
                        eng.tensor_add(ytile[:, 0:4, lo:hi],
                                       ytile[:, 0:4, lo:hi],
                                       ytile[:, 4:8, lo:hi])
                        eng.tensor_add(ytile[:, 0:2, lo:hi],
                                       ytile[:, 0:2, lo:hi],
                                       ytile[:, 2:4, lo:hi])
                        eng.tensor_add(ytile[:, 0, lo:hi],
                                       ytile[:, 0, lo:hi],
                                       ytile[:, 1, lo:hi])
                        eng.tensor_add(acc[:, lo:hi], acc[:, lo:hi],
                                       ytile[:, 0, lo:hi])
            nc.tensor.matmul(bc[:], ones128[:], acc[:],
                             start=True, stop=True)

            # ---- phase B: partial dots for ALL rows while x streams ----
            s_part = small.tile([128, NT], FP32)
            cc_in = dram.tile([B], FP32)
            cc_out = dram.tile([B], FP32)
            for s in range(NST):
                xtile = xpool.tile([128, NSUB, FL], FP32, tag="x")
                cut = NSUB // 2 if s < NST - 1 else 6
                nc.sync.dma_start(xtile[:, 0:cut, :], x_r[s, :, 0:cut, :])
                nc.scalar.dma_start(xtile[:, cut:, :], x_r[s, :, cut:, :])
                for t in range(NSUB):
                    m = s * NSUB + t
                    prod = scratch.tile([128, FL], FP32, tag="sc")
                    nc.vector.scalar_tensor_tensor(
                        out=prod[:],
                        in0=xtile[:, t, :],
                        scalar=1.0,
                        in1=bc[:],
                        op0=mybir.AluOpType.mult,
                        op1=mybir.AluOpType.mult,
                        accum_out=s_part[:, m:m + 1],
                    )
                # bounce finished partial columns early so only a 4KB
                # chunk trails the final dot
                if s == 3:
                    nc.gpsimd.dma_start(
                        cc_in[:].rearrange("(p m) -> p m", p=128)[:, 0:32],
                        s_part[:, 0:32])
                elif s == 6:
                    nc.gpsimd.dma_start(
                        cc_in[:].rearrange("(p m) -> p m", p=128)[:, 32:56],
                        s_part[:, 32:56])

            # ---- ONE post-stream A2A of all partials (p-major);
            # the final 4KB chunk goes on the (idle) sync HWDGE queue,
            # whose completion latency beats SWDGE ----
            nc.sync.dma_start(
                cc_in[:].rearrange("(p m) -> p m", p=128)[:, 56:64],
                s_part[:, 56:64])
            nc.gpsimd.collective_compute(
                "AllToAll",
                mybir.AluOpType.bypass,
                replica_groups=[list(range(NCORES))],
                ins=[cc_in.opt()],
                outs=[cc_out.opt()],
            )

            # ---- tail: fold 8 shards (own 1024 rows), hardswish, out ----
            # A2A out[k]: k = 1024*j + 64*pl + m -> (pl, j, m); element
            # (pl, m) after the j-fold is global row 128m + 16r + pl
            st = small.tile([16, NT], FP32)
            fa = small.tile([16, NSUB, NT], FP32, tag="fa")
            cc_or = cc_out[:].rearrange("(j pl m) -> pl j m", pl=16, m=NT)
            nc.sync.dma_start(fa[:, 0:4, :], cc_or[:, 0:4, :])
            nc.scalar.dma_start(fa[:, 4:8, :], cc_or[:, 4:8, :])
            nc.vector.tensor_add(fa[:, 0:4, :], fa[:, 0:4, :], fa[:, 4:8, :])
            nc.vector.tensor_add(fa[:, 0:2, :], fa[:, 0:2, :], fa[:, 2:4, :])
            nc.vector.tensor_tensor(
                out=st[:], in0=fa[:, 0, :], in1=fa[:, 1, :],
                op=mybir.AluOpType.add)
            t_ = small.tile([16, NT], FP32)
            nc.vector.tensor_scalar(
                out=t_[:], in0=st[:], scalar1=3.0, scalar2=0.0,
                op0=mybir.AluOpType.add, op1=mybir.AluOpType.max,
            )
            nc.vector.tensor_scalar(
                out=t_[:], in0=t_[:], scalar1=6.0, scalar2=1.0 / 6.0,
                op0=mybir.AluOpType.min, op1=mybir.AluOpType.mult,
            )
            r = small.tile([16, NT], FP32)
            nc.vector.tensor_tensor(
                out=r[:], in0=st[:], in1=t_[:], op=mybir.AluOpType.mult,
            )
            nc.vector.tensor_tensor(
                out=r[:], in0=r[:], in1=noise_t[:], op=mybir.AluOpType.add,
            )
            nc.vector.tensor_scalar(
                out=r[:], in0=r[:], scalar1=-0.5, scalar2=0.5,
                op0=mybir.AluOpType.max, op1=mybir.AluOpType.min,
            )
            nc.sync.dma_start(out_d[:, :], r[:])

    nc.compile()
    return nc


def _get_nc():
    if "nc" not in _CACHE:
        _CACHE["nc"] = _build()
    return _CACHE["nc"]


# device row (s p c) -> global row 128*(8s+c)+p, so that partials column
# m = 8s+c, partition p lands at global row 128m+p
def _permute_rows(a: np.ndarray) -> np.ndarray:
    # a: (8192, cols); view as (s, c, p, cols), want (s, p, c, cols)
    return np.ascontiguousarray(
        a.reshape(NST, NSUB, 128, a.shape[1]).transpose(0, 2, 1, 3)
        .reshape(B, a.shape[1])
    )


def kernel(x: np.ndarray, y: np.ndarray, noise: np.ndarray, **_run_kwargs) -> np.ndarray:
    x = np.ascontiguousarray(x, dtype=np.float32)
    y = np.ascontiguousarray(y, dtype=np.float32)
    noise = np.ascontiguousarray(noise, dtype=np.float32)

    nc = _get_nc()
    xp = _permute_rows(x)
    # noise for core r in (16, 64) layout: element (pl, m) = global row
    # 128m + 16r + pl
    nz = noise[:, 0].reshape(NT, 128).T     # (128, 64): (p, m)
    in_maps = [
        {
            "x": np.ascontiguousarray(xp[:, i * FL:(i + 1) * FL]),
            "y": np.ascontiguousarray(y[:, i * FL:(i + 1) * FL]),
            "noise": np.ascontiguousarray(nz[16 * i:16 * (i + 1), :]),
        }
        for i in range(NCORES)
    ]
    res = run_bass_kernel_spmd(nc, in_maps, list(range(NCORES)), **_run_kwargs)
    # core r's out (16, 64): element (pl, m) = global row 128m + 16r + pl
    stacked = np.stack([res.results[i]["out"] for i in range(NCORES)])  # (8,16,64)
    out = stacked.transpose(2, 0, 1).reshape(B, 1)                      # (64*8*16,1)
    if _run_kwargs:
        _CACHE["last_results"] = res
    return out
